# revision 74
# baseline (speedup 1.0000x reference)
"""Trainium2 Bass kernel for causal (strict-future-masked) MHA + residual + LayerNorm.

Reference semantics (Keras MultiHeadAttention, inference):
    q,k,v = einsum(x, W{q,k,v}) + b    [B,S,H,DH]
    scores = q.k / sqrt(DH); mask allows j > i (STRICT UPPER triangle);
    masked entries get -1e9 added (in fp32 this makes the fully-masked row
    S-1 collapse to exactly -1e9 -> uniform softmax = 1/S).
    ctx = probs @ v; out = ctx @ Wo + bo; y = LN(x + out) * gamma + beta.

Shapes: B=2, S=2048, D=1024, H=16, DH=64.

Sharding (8 cores): core c -> batch b = c//4, head-group hg = c%4 (4 heads),
RS rank r = c%4. Each core computes q/k/v + attention + out-proj partial for
its 4 heads over the full sequence, ReduceScatter([2048,1024] bf16) within
its 4-core batch group yields rows [512r, 512r+512) of the head-summed
attn_out, then residual + LayerNorm locally. Host assembles 8 x [512,1024].

Device-side design (v2 -- exp-stream-critical schedule):
  The scalar (Activation) engine's exp stream (~69K cols @ 0.83ns/col) is
  the critical resource; everything else is organized to hide under it.
  - QKV via fp8e4 DoubleRow matmuls (x and W quantized fp8 on host, pairs
    of 128-row d-blocks per instruction at 0.5 cyc/col).
  - qT/kT stored bf16 [128 = 2 heads x 64dh, S]; v stored fp8 in even/odd
    kb-pair tiles [128, 2, 4x(64+onescol)] (ones col -> Z row via matmul).
  - Rows processed qb = 3,2,1,0 (strict-upper mask: row qb needs only KV
    windows >= qb) interleaved with JIT KV/Q projections; the exp stream
    starts a few us in and everything else hides under it.
  - Causal masking: -240 band add-matmuls on the PE into score PSUM
    (lhsT = lower-tri(-240), rhs = I / [I|I]); exp underflows to exact 0
    in fp8. No vector-engine masking.
  - E tiles fp8e4 (exp bias -ln8 keeps E <= ~40 << 240-max); full kb
    blocks pair into [128, 2, 512] tiles; the rr3/rr2 diagonals pair too
    (rr2 zero-padded via one memset) -> ctx mostly fp8 DoubleRow.
  - Out-proj fp8 DoubleRow over t2-pairs; stage -> DRAM bf16; chunked
    ReduceScatter per q-block overlaps the remaining attention.
  - DMAs are merged into few large transfers (one per xt window, one
    weight wall, one const wall, one xres, one stage per qb, one out) and
    all pure loads issue up front: the cost model charges ~625ns of a
    shared HWDGE port per DMA and the in-order queues otherwise propagate
    store-waits into load stalls.
  - LayerNorm: sums/var on DVE mid-stream; Sqrt lives in a different
    activation table than Exp, so the 4 sqrt+scale finals are deferred
    past the last exp (single table switch).
"""

import numpy as np

B, S, D, H, DH = 2, 2048, 1024, 16, 64
HPC = 4            # heads per core
NCORES = 8
QB = 512           # q-block
NQB = S // QB      # 4
KBLK = 128         # kpos block
NKB = S // KBLK    # 16
SCALE = 1.0 / 8.0  # 1/sqrt(DH)
LN8 = 2.0794415416798357  # ln(8): exp bias; E = exp(s/8 - ln8) <= ~40
EPS = 1.0e-6
MASKC = -240.0     # band-add constant; exp((s-240)/8 - ln8) -> fp8 0
# Schraudolph fast-exp (DVE/Pool offload): int32(SA*s + SB) bitcast to f32
# approximates exp(s/8 - ln8) with ~2% rms error; the Z normalization is
# computed from the same E so the bias largely cancels in the softmax.
SEXP_A = 12102203.161561485 / 8.0
SEXP_B = float(127 * (1 << 23) - 486411 - 3 * (1 << 23))
# full-block pairs (per head, by index in the pair list) offloaded to DVE
SEXP_DVE = {3: (), 2: (), 1: (), 0: ()}
SEXP_C_DVE = True  # C diagonal units' fast-exp on DVE

_CACHE = {}


def _build_program(with_collective=True, ln_affine=False, qkv_bias=False):
    """Build + compile the SPMD Bass program (identical on all 8 cores)."""
    import concourse.bass as bass
    import concourse.tile as tile
    from concourse import bacc, mybir

    f32 = mybir.dt.float32
    f32r = mybir.dt.float32r
    i32 = mybir.dt.int32
    bf16 = mybir.dt.bfloat16
    fp8 = mybir.dt.float8e4
    Alu = mybir.AluOpType
    Act = mybir.ActivationFunctionType
    DR = mybir.MatmulPerfMode.DoubleRow

    nc = bacc.Bacc("TRN2", target_bir_lowering=False, debug=False,
                   num_devices=NCORES)

    # ---- external I/O ----
    # xt8: per window one contiguous [128, 4kcp, 2, 512] fp8 block
    xt8 = nc.dram_tensor("xt8", [NQB, 128, 4, 2, 512], fp8,
                         kind="ExternalInput").ap()
    # weights as separate tensors, DMA'd in criticality order:
    # wk before xt3 (first K proj), wq right after (first Q proj), then wv,
    # wo much later (first needed by out_proj(2))
    wk_d = nc.dram_tensor("wk8", [128, 2048], fp8, kind="ExternalInput").ap()
    wq_d = nc.dram_tensor("wq8", [128, 2048], fp8, kind="ExternalInput").ap()
    wv_d = nc.dram_tensor("wv8", [128, 2560], fp8, kind="ExternalInput").ap()
    wo_d = nc.dram_tensor("wo8", [128, 2048], fp8, kind="ExternalInput").ap()
    # const wall: maskA [128,128] | maskI [128,128] | maskII [128,256]
    # | ones8 [128,2] -> [128, 514] fp8
    cwall = nc.dram_tensor("cwall", [128, 800], fp8,
                           kind="ExternalInput").ap()
    xres = nc.dram_tensor("xres", [QB, D], f32, kind="ExternalInput").ap()
    if qkv_bias:
        onesr_d = nc.dram_tensor("onesr32", [1, 512], f32,
                                 kind="ExternalInput").ap()
        bq_d = nc.dram_tensor("bq_r", [1, 256], f32, kind="ExternalInput").ap()
        bk_d = nc.dram_tensor("bk_r", [1, 256], f32, kind="ExternalInput").ap()
        bv_d = nc.dram_tensor("bv_r", [1, 320], f32, kind="ExternalInput").ap()
    if ln_affine:
        gamma_r = nc.dram_tensor("gamma_r", [1, D], f32,
                                 kind="ExternalInput").ap()
        beta_r = nc.dram_tensor("beta_r", [1, D], f32,
                                kind="ExternalInput").ap()
    out = nc.dram_tensor("out", [QB, D], f32, kind="ExternalOutput").ap()

    # internal DRAM for the chunked collectives (one per q-block)
    attn_dram_l = [nc.dram_tensor(f"attn_dram{j}", [QB, D], bf16)
                   for j in range(NQB)]
    rs_dram_l = [nc.dram_tensor(f"rs_dram{j}", [128, D], bf16)
                 for j in range(NQB)]

    ROWS = [3, 2, 1, 0]

    with tile.TileContext(nc) as tc, \
         nc.allow_low_precision(reason="fp8/bf16 attention path"):
        from contextlib import ExitStack
        with ExitStack() as ctx:
            # ---------- persistent pools ----------
            p_const = ctx.enter_context(tc.tile_pool(name="const", bufs=1))
            p_w = ctx.enter_context(tc.tile_pool(name="w", bufs=1))
            p_qk = ctx.enter_context(tc.tile_pool(name="qk", bufs=1))
            p_v = ctx.enter_context(tc.tile_pool(name="v", bufs=1))
            p_ctx = ctx.enter_context(tc.tile_pool(name="ctxp", bufs=1))
            p_xt = ctx.enter_context(tc.tile_pool(name="xt", bufs=1))
            p_e = ctx.enter_context(tc.tile_pool(name="e", bufs=8))
            p_es = ctx.enter_context(tc.tile_pool(name="es", bufs=4))
            p_z = ctx.enter_context(tc.tile_pool(name="z", bufs=4))
            p_bcn = ctx.enter_context(tc.tile_pool(name="bcn", bufs=3))
            p_stage = ctx.enter_context(tc.tile_pool(name="stage", bufs=2))
            p_lnp = ctx.enter_context(tc.tile_pool(name="lnp", bufs=1))
            p_lns = ctx.enter_context(tc.tile_pool(name="lns", bufs=2))
            p_lnc = ctx.enter_context(tc.tile_pool(name="lnc", bufs=10))
            ps_kv = ctx.enter_context(
                tc.tile_pool(name="ps_kv", bufs=2, space="PSUM"))
            ps_s = ctx.enter_context(
                tc.tile_pool(name="ps_s", bufs=2, space="PSUM"))
            ps_cu = ctx.enter_context(
                tc.tile_pool(name="ps_cu", bufs=2, space="PSUM"))

            # ---------- merged loads, all issued up front (SP queue) -------
            wkt = p_w.tile([128, 2048], fp8, name="wkt", tag="wkt")
            nc.sync.dma_start(wkt[:], wk_d[:])
            wk_sb = wkt.rearrange("p (k t m) -> p k t m", k=4, t=2)

            xt_sb = {}
            t = p_xt.tile([128, 4, 2, 512], fp8, name="xt3", tag="xt3")
            nc.sync.dma_start(t[:], xt8[3])
            xt_sb[3] = t

            wqt = p_w.tile([128, 2048], fp8, name="wqt", tag="wqt")
            nc.sync.dma_start(wqt[:], wq_d[:])
            wq_sb = wqt.rearrange("p (k t m) -> p k t m", k=4, t=2)

            # masks are first needed by the band adds at ~8us; the warmup
            # runs on a memset tile, so this load can follow the projections
            cw = p_const.tile([128, 800], fp8, name="cw", tag="cw")
            nc.sync.dma_start(cw[:], cwall[:])
            maskA = cw[:, 0:128]
            maskI = cw[:, 128:256]
            maskII = cw[:, 256:512]
            ones8 = cw[:, 512:544].rearrange("p (t o) -> p t o", o=16)
            negc_row = cw[0:1, 544:672]
            ones_row = cw[0:1, 672:800]

            wvt = p_w.tile([128, 2560], fp8, name="wvt", tag="wvt")
            nc.sync.dma_start(wvt[:], wv_d[:])
            wv_sb = wvt.rearrange("p (k t m) -> p k t m", k=4, t=2)

            for w in (2, 1, 0):
                t = p_xt.tile([128, 4, 2, 512], fp8, name=f"xt{w}",
                              tag=f"xt{w}")
                nc.sync.dma_start(t[:], xt8[w])
                xt_sb[w] = t

            wot = p_w.tile([128, 2048], fp8, name="wot", tag="wot")
            nc.sync.dma_start(wot[:], wo_d[:])
            wo_ap = wot.rearrange("p (t m) -> p t m", t=2)

            # xres: one DMA into [128, 4, 1024] (row chunk qb on axis 1)
            xr_all = p_lnp.tile([128, 4, D], f32, name="xr_all", tag="xr")
            nc.sync.dma_start(
                xr_all[:], xres.rearrange("(q p) d -> p q d", p=128))

            one32 = p_const.tile([1, 1], f32, name="one32", tag="one32")
            nc.vector.memset(one32[:], 1.0)
            eps_col = p_const.tile([128, 1], f32, name="eps_col", tag="eps")
            nc.vector.memset(eps_col[:], EPS)
            ln8_col = p_const.tile([128, 1], f32, name="ln8_col", tag="ln8")
            nc.vector.memset(ln8_col[:], -LN8)

            if qkv_bias:
                onesr = p_const.tile([1, 512], f32, name="onesr", tag="onesr")
                nc.sync.dma_start(onesr[:], onesr_d[:])
                bq_row = p_const.tile([1, 256], f32, name="bq_row", tag="bqr")
                nc.sync.dma_start(bq_row[:], bq_d[:])
                bk_row = p_const.tile([1, 256], f32, name="bk_row", tag="bkr")
                nc.sync.dma_start(bk_row[:], bk_d[:])
                bv_row = p_const.tile([1, 320], f32, name="bv_row", tag="bvr")
                nc.sync.dma_start(bv_row[:], bv_d[:])
            if ln_affine:
                gamma_row = p_const.tile([1, D], f32, name="gamma_row",
                                         tag="gr")
                nc.sync.dma_start(gamma_row[:], gamma_r[:])
                beta_row = p_const.tile([1, D], f32, name="beta_row",
                                        tag="br")
                nc.sync.dma_start(beta_row[:], beta_r[:])

            # persistent activations
            qT_sb = [p_qk.tile([128, S], bf16, name=f"qT{t2}", tag=f"qT{t2}")
                     for t2 in range(2)]
            kT_sb = [p_qk.tile([128, S], bf16, name=f"kT{t2}", tag=f"kT{t2}")
                     for t2 in range(2)]
            # v pair tiles: kp pairs (2kp, 2kp+1); cols 65*hi..65*hi+64 + ones
            # head stride 80 = 5x16B: dual-fp8 Ldweights requires outer AP
            # steps 16B-aligned (cols 80h..80h+63 = weights, 80h+64 = ones)
            vp_sb = [p_v.tile([128, 2, 320], fp8, name=f"vp{kp}",
                              tag=f"vp{kp}") for kp in range(8)]
            ctx_sb = p_ctx.tile([128, 2, S], fp8, name="ctxT", tag="ctxT")

            # LN persistent tiles (finals deferred past the last exp)
            y32_sb = {qb: p_lnp.tile([128, D], f32, name=f"y32_{qb}",
                                     tag=f"y32_{qb}") for qb in ROWS}
            varc_sb = {qb: p_lnp.tile([128, 1], f32, name=f"varc{qb}",
                                      tag=f"varc{qb}") for qb in ROWS}
            negmu_sb = {qb: p_lnp.tile([128, 1], f32, name=f"negmu{qb}",
                                       tag=f"negmu{qb}") for qb in ROWS}

            if ln_affine:
                gamma_bc = p_const.tile([128, D], f32, name="gamma_bc",
                                        tag="gbc")
                nc.gpsimd.partition_broadcast(gamma_bc[:], gamma_row[:])
                beta_bc = p_const.tile([128, D], f32, name="beta_bc",
                                       tag="bbc")
                nc.gpsimd.partition_broadcast(beta_bc[:], beta_row[:])

            def k_proj(w):
                xt_w = xt_sb[w]
                # K for window w -> kT bf16 (copies on Pool engine)
                for t2 in range(2):
                    acc = ps_kv.tile([128, 512], f32, name="kvp", tag="kvp")
                    for kcp in range(4):
                        nc.tensor.matmul(
                            acc[:],
                            wk_sb[:, kcp, :, 128 * t2:128 * t2 + 128],
                            xt_w[:, kcp, :, :], perf_mode=DR,
                            start=(kcp == 0),
                            stop=(kcp == 3 and not qkv_bias))
                    if qkv_bias:
                        nc.tensor.matmul(
                            acc[:], bk_row[0:1, 128 * t2:128 * t2 + 128],
                            onesr[:], start=False, stop=True)
                    # DVE: the gpsimd library load blocks the Pool queue
                    # for ~7us at startup, and Pool copies are slower
                    nc.vector.tensor_copy(
                        kT_sb[t2][:, 512 * w:512 * w + 512], acc[:])

            def v_proj(w):
                xt_w = xt_sb[w]
                # V for window w -> fp8 pair tiles (copies on DVE)
                for tsub in range(4):
                    kb = 4 * w + tsub
                    kp, half = kb // 2, kb % 2
                    acc = ps_kv.tile([128, 320], f32, name="vpp", tag="kvp")
                    for kcp in range(4):
                        nc.tensor.matmul(
                            acc[:],
                            xt_w[:, kcp, :, 128 * tsub:128 * tsub + 128],
                            wv_sb[:, kcp, :, :], perf_mode=DR,
                            start=(kcp == 0),
                            stop=(kcp == 3 and not qkv_bias))
                    if qkv_bias:
                        nc.tensor.matmul(
                            acc[:], onesr[0:1, 0:128], bv_row[:],
                            start=False, stop=True)
                    nc.vector.tensor_copy(vp_sb[kp][:, half, :], acc[:])
                # ones cols for the Z-row trick (after both halves land)
                for kp in (2 * w, 2 * w + 1):
                    vcols = vp_sb[kp].rearrange("p t (h e) -> p t h e", e=80)
                    nc.vector.memset(vcols[:, :, :, 64:65], 1.0)

            def q_proj(qb):
                xt_w = xt_sb[qb]
                for t2 in range(2):
                    acc = ps_kv.tile([128, 512], f32, name="qp", tag="kvp")
                    for kcp in range(4):
                        nc.tensor.matmul(
                            acc[:],
                            wq_sb[:, kcp, :, 128 * t2:128 * t2 + 128],
                            xt_w[:, kcp, :, :], perf_mode=DR,
                            start=(kcp == 0),
                            stop=(kcp == 3 and not qkv_bias))
                    if qkv_bias:
                        nc.tensor.matmul(
                            acc[:], bq_row[0:1, 128 * t2:128 * t2 + 128],
                            onesr[:], start=False, stop=True)
                    nc.vector.tensor_copy(
                        qT_sb[t2][:, 512 * qb:512 * qb + 512], acc[:])

            def score_mm(sT_roi, t2, po, kb, qoff, w, start, stop):
                nc.tensor.matmul(
                    sT_roi,
                    kT_sb[t2][po:po + 64, 128 * kb:128 * kb + 128],
                    qT_sb[t2][po:po + 64, qoff:qoff + w],
                    start=start, stop=stop, skip_group_check=True)

            def band_add(sT_roi, rhs, stop):
                nc.tensor.matmul(sT_roi, maskA, rhs,
                                 start=False, stop=stop,
                                 skip_group_check=True)

            def exp_to(e_roi, s_roi):
                nc.scalar.activation(e_roi, s_roi, Act.Exp,
                                     scale=SCALE, bias=ln8_col[:])

            def attn_row(qb, post=None):
                # ctx matmuls are emitted one unit LATE: the in-order PE
                # queue then issues the next unit's score matmuls while the
                # current unit's exps run, keeping the Act stream gapless.
                qoff = QB * qb
                kb0 = 4 * qb
                pend = [None]

                def emit(scores_fn, ctx_fn):
                    scores_fn()
                    if pend[0] is not None:
                        pend[0]()
                    pend[0] = ctx_fn

                for t2 in range(2):
                    for half in range(2):
                        po = 64 * half
                        hi = 2 * t2 + half
                        ctxu = ps_cu.tile([65, QB], f32, name="ctxu",
                                          tag="cu")
                        vsl = slice(80 * hi, 80 * hi + 65)
                        first = [True]

                        def mk_pair(kp, ctxu=ctxu, t2=t2, po=po, vsl=vsl,
                                    first=first):
                            ep = p_e.tile([128, 2, 512], fp8, name="ep",
                                          tag="ep")

                            def scores():
                                sT2 = ps_s.tile([128, 2, QB], f32,
                                                name="sT2", tag="sT")
                                for j in range(2):
                                    score_mm(sT2[:, j, :], t2, po,
                                             2 * kp + j, qoff, QB,
                                             True, True)
                                exp_to(ep[:], sT2[:])

                            def ctx():
                                nc.tensor.matmul(
                                    ctxu[:], vp_sb[kp][:, :, vsl], ep[:],
                                    perf_mode=DR, start=first[0],
                                    stop=False, skip_group_check=True)
                                first[0] = False
                            return scores, ctx

                        def mk_pair_dve(kp, ctxu=ctxu, t2=t2, po=po,
                                        vsl=vsl, first=first):
                            # Schraudolph fast-exp on the DVE; E stays f32
                            # bits (bitcast f32r), ctx via two singles
                            es = p_es.tile([128, 2, 512], i32, name="es",
                                           tag="es")

                            def scores():
                                for j in range(2):
                                    sT = ps_s.tile([128, QB], f32,
                                                   name="sT", tag="sT")
                                    score_mm(sT[:], t2, po, 2 * kp + j,
                                             qoff, QB, True, True)
                                    nc.vector.tensor_scalar(
                                        es[:, j, :], sT[:], SEXP_A, SEXP_B,
                                        Alu.mult, Alu.add)

                            def ctx():
                                for j in range(2):
                                    nc.tensor.matmul(
                                        ctxu[:], vp_sb[kp][:, j, vsl],
                                        es[:, j, :].bitcast(f32r),
                                        start=(first[0] and j == 0),
                                        stop=False, skip_group_check=True)
                                first[0] = False
                            return scores, ctx

                        def mk_ab(ctxu=ctxu, t2=t2, po=po, vsl=vsl,
                                  first=first):
                            # A = rr2 @ [0,384) zero-padded to 512 (memset),
                            # B = rr3 @ [0,512); v pair (rr2, rr3) =
                            # vp[2qb+1] halves (0, 1) in natural order.
                            eab = p_e.tile([128, 2, 512], fp8, name="eab",
                                           tag="ep")

                            def scores():
                                sT2 = ps_s.tile([128, 2, QB], f32,
                                                name="sT2", tag="sT")
                                score_mm(sT2[:, 0, 0:384], t2, po, kb0 + 2,
                                         qoff, 384, True, False)
                                band_add(sT2[:, 0, 256:384], maskI, False)
                                # fill the rr2 pad [384,512) with -240 (on
                                # the zeroed bank) so one merged exp covers
                                # both banks and lands exact fp8 zeros there
                                nc.tensor.matmul(
                                    sT2[:, 0, 384:512], negc_row, ones_row,
                                    start=False, stop=True,
                                    skip_group_check=True)
                                score_mm(sT2[:, 1, :], t2, po, kb0 + 3,
                                         qoff, QB, True, False)
                                band_add(sT2[:, 1, 384:512], maskI, True)
                                exp_to(eab[:], sT2[:])

                            def ctx():
                                nc.tensor.matmul(
                                    ctxu[:], vp_sb[2 * qb + 1][:, :, vsl],
                                    eab[:], perf_mode=DR, start=first[0],
                                    stop=False, skip_group_check=True)
                                first[0] = False
                            return scores, ctx

                        def mk_c(ctxu=ctxu, t2=t2, po=po, vsl=vsl):
                            # C: rr1 @ [0,256) + rr0 @ [256,384); bands via
                            # one [I|I] add at [128,384). Optionally
                            # Schraudolph on GpSimd (masked entries land at
                            # ~1e-14 instead of exact 0 -- negligible).
                            if SEXP_C_DVE:
                                eC = p_es.tile([128, 384], i32, name="eC",
                                               tag="es")
                            else:
                                eC = p_e.tile([128, 384], fp8, name="eC",
                                              tag="ep")

                            def scores():
                                sTw = ps_s.tile([128, 2, QB], f32,
                                                name="sTc", tag="sT")
                                sT = sTw[:, 0, 0:384]
                                score_mm(sT[:, 0:256], t2, po, kb0 + 1,
                                         qoff, 256, True, False)
                                score_mm(sT[:, 256:384], t2, po, kb0,
                                         qoff, 128, False, False)
                                band_add(sT[:, 128:384], maskII, True)
                                if SEXP_C_DVE:
                                    nc.vector.tensor_scalar(
                                        eC[:], sT[:], SEXP_A, SEXP_B,
                                        Alu.mult, Alu.add)
                                else:
                                    exp_to(eC[:], sT[:])

                            def ctx():
                                e1 = eC[:, 0:256]
                                e0 = eC[:, 256:384]
                                if SEXP_C_DVE:
                                    e1 = e1.bitcast(f32r)
                                    e0 = e0.bitcast(f32r)
                                nc.tensor.matmul(
                                    ctxu[:, 0:256], vp_sb[2 * qb][:, 1, vsl],
                                    e1, start=False, stop=False,
                                    skip_group_check=True)
                                nc.tensor.matmul(
                                    ctxu[:, 0:128], vp_sb[2 * qb][:, 0, vsl],
                                    e0, start=False, stop=True,
                                    skip_group_check=True)
                            return scores, ctx

                        for pi, kp in enumerate(range(2 * qb + 2, 8)):
                            if pi in SEXP_DVE[qb]:
                                emit(*mk_pair_dve(kp))
                            else:
                                emit(*mk_pair(kp))
                        emit(*mk_ab())
                        emit(*mk_c())

                        def norm(ctxu=ctxu, t2=t2, po=po, qoff=qoff,
                                 half=half):
                            # ctx = ctxu[0:64] / Z  (Z = row 64). The very
                            # last head of the last row is normalized in
                            # 128-col chunks so out_proj qt0 (the LN-chain
                            # gate) starts ~1.5us earlier.
                            if qb == 0 and t2 == 1 and half == 1:
                                for cc in range(4):
                                    cs = slice(128 * cc, 128 * cc + 128)
                                    zinv = p_z.tile([1, 128], f32,
                                                    name="zinv", tag="zinv")
                                    nc.vector.reciprocal(
                                        zinv[:], ctxu[64:65, cs])
                                    zbs = p_bcn.tile([64, 128], f32,
                                                     name="zbs", tag="zbs")
                                    nc.gpsimd.partition_broadcast(
                                        zbs[:], zinv[:])
                                    nc.vector.tensor_mul(
                                        ctx_sb[po:po + 64, t2,
                                               qoff + 128 * cc:
                                               qoff + 128 * cc + 128],
                                        ctxu[0:64, cs], zbs[:])
                                return
                            zinv = p_z.tile([1, QB], f32, name="zinv",
                                            tag="zinv")
                            nc.vector.reciprocal(zinv[:], ctxu[64:65, :])
                            zbs = p_bcn.tile([64, QB], f32, name="zbs",
                                             tag="zbs")
                            nc.gpsimd.partition_broadcast(zbs[:], zinv[:])
                            nc.vector.tensor_mul(
                                ctx_sb[po:po + 64, t2, qoff:qoff + QB],
                                ctxu[0:64, :], zbs[:])

                        prev_ctx = pend[0]

                        def flush_and_norm(prev_ctx=prev_ctx, norm=norm):
                            prev_ctx()
                            norm()
                        pend[0] = flush_and_norm
                        if post is not None and t2 == 0 and half == 0:
                            # deferred work from the previous row: its PE
                            # burst lands inside this row's exp stream
                            # (head 0 is already buffered ahead)
                            post()
                            post = None
                if pend[0] is not None:
                    pend[0]()

            def out_proj(qb):
                # qt0 gets its own tile + DMA with plain 2D access patterns:
                # the LN chain (and the timed-path y read) depends only on
                # rows 0:128, so this slice must move with minimum latency.
                stage0 = p_stage.tile([128, D], bf16, name="stage0",
                                      tag="stage0")
                stage = p_stage.tile([128, 3, D], bf16, name="stage",
                                     tag="stage")
                ad = attn_dram_l[qb].rearrange("(t p) d -> p t d", p=128)
                for qtl in range(4):
                    toff = QB * qb + 128 * qtl
                    for ob in range(2):
                        acc = ps_cu.tile([128, 512], f32, name="op", tag="cu")
                        nc.tensor.matmul(
                            acc[:], ctx_sb[:, :, toff:toff + 128],
                            wo_ap[:, :, 512 * ob:512 * ob + 512],
                            perf_mode=DR, start=True, stop=True)
                        # PSUM readers are DVE/Act only. For qb0 the Act
                        # engine is idle (exp stream over) and Copy is in
                        # the exp act-table: split copies Act/DVE there.
                        dst = (stage0[:, 512 * ob:512 * ob + 512] if qtl == 0
                               else stage[:, qtl - 1,
                                          512 * ob:512 * ob + 512])
                        if qb == 0 and ob == 1:
                            nc.scalar.copy(dst, acc[:])
                        else:
                            nc.vector.tensor_copy(dst, acc[:])
                    if qtl == 0:
                        nc.sync.dma_start(attn_dram_l[qb][0:128, :],
                                          stage0[:])
                nc.sync.dma_start(ad[:, 1:4, :], stage[:])
                if with_collective:
                    nc.gpsimd.collective_compute(
                        "ReduceScatter",
                        mybir.AluOpType.add,
                        replica_groups=[[0, 1, 2, 3], [4, 5, 6, 7]],
                        ins=[attn_dram_l[qb][:]],
                        outs=[rs_dram_l[qb][:]],
                    )


            def ln_sums(qb):
                # residual add + sum/sq-sum (DVE); var = sqsum/D - mu^2 via
                # column algebra; sqrt/scale deferred past the last exp
                yb = p_lns.tile([128, D], bf16, name="yb", tag="yb")
                # real path: read the collective's output; timed path: read
                # the local partial directly (the RS itself is accounted by
                # the fixed tail constant, not the cost model)
                ysrc = (rs_dram_l[qb][:] if with_collective
                        else attn_dram_l[qb][0:128, :])
                nc.gpsimd.dma_start(yb[:], ysrc)
                y32 = y32_sb[qb]
                # TensorScalarPtr-class ops (stt with tensor operand) are
                # DVE-only on real hardware
                eng = nc.vector
                ysum = p_lnc.tile([128, 1], f32, name="ysum", tag="lncol")
                eng.scalar_tensor_tensor(
                    y32[:], yb[:], 1.0, xr_all[:, qb, :], Alu.mult, Alu.add,
                    accum_out=ysum[:])
                negmu = negmu_sb[qb]
                nc.vector.tensor_scalar_mul(negmu[:], ysum[:],
                                            -1.0 / float(D))
                sq = p_lns.tile([128, D], f32, name="sq", tag="sq")
                sqsum = p_lnc.tile([128, 1], f32, name="sqsum", tag="lncol")
                eng.scalar_tensor_tensor(
                    sq[:], y32[:], 1.0, y32[:], Alu.mult, Alu.mult,
                    accum_out=sqsum[:])
                musq = p_lnc.tile([128, 1], f32, name="musq", tag="lncol")
                nc.vector.tensor_tensor(musq[:], negmu[:], negmu[:],
                                        Alu.mult)
                # AP-scalar (Ptr) tensor_scalar is DVE-only on real hw
                nc.vector.tensor_scalar(varc_sb[qb][:], sqsum[:],
                                        1.0 / float(D), musq[:],
                                        Alu.mult, Alu.subtract)

            def ln_final(qb):
                # rstd = rsqrt(var + eps) fully on the DVE (Quake-style
                # bit-trick init + 2 Newton steps on [128,1] columns, ~1e-6
                # rel err). Keeps the Activation engine exp-only: the tile
                # scheduler otherwise interleaves Sqrt activations into the
                # exp stream at 2x1283ns act-table loads apiece.
                ve = p_lnc.tile([128, 1], f32, name="ve", tag="lncol")
                nc.vector.tensor_scalar_add(ve[:], varc_sb[qb][:], EPS)
                i1 = p_lnc.tile([128, 1], i32, name="i1", tag="lncol")
                nc.vector.tensor_scalar(i1[:], ve.bitcast(i32)[:], 1,
                                        -1, Alu.logical_shift_right,
                                        Alu.bitwise_xor)
                u0 = p_lnc.tile([128, 1], f32, name="u0", tag="lncol")
                nc.vector.tensor_scalar_add(u0.bitcast(i32)[:], i1[:],
                                            0x5F3759E0)
                u = u0
                for it in range(2):
                    a = p_lnc.tile([128, 1], f32, name="nta", tag="lncol")
                    nc.vector.tensor_tensor(a[:], u[:], u[:], Alu.mult)
                    b = p_lnc.tile([128, 1], f32, name="ntb", tag="lncol")
                    nc.vector.tensor_tensor(b[:], a[:], ve[:], Alu.mult)
                    c = p_lnc.tile([128, 1], f32, name="ntc", tag="lncol")
                    nc.vector.tensor_scalar(c[:], b[:], -0.5, 1.5,
                                            Alu.mult, Alu.add)
                    un = p_lnc.tile([128, 1], f32, name="ntu", tag="lncol")
                    nc.vector.tensor_tensor(un[:], u[:], c[:], Alu.mult)
                    u = un
                rstd = u
                mr = p_lnc.tile([128, 1], f32, name="mr", tag="lncol")
                nc.vector.tensor_tensor(mr[:], negmu_sb[qb][:], rstd[:],
                                        Alu.mult)
                yout = p_lns.tile([128, D], f32, name="yout", tag="yout")
                # AP-scalar (Ptr) tensor_scalar is DVE-only on real hw
                nc.vector.tensor_scalar(yout[:], y32_sb[qb][:], rstd[:],
                                        mr[:], Alu.mult, Alu.add)
                if ln_affine:
                    nc.vector.scalar_tensor_tensor(
                        yout[:], yout[:], 1.0, gamma_bc[:],
                        Alu.mult, Alu.mult)
                    nc.vector.tensor_add(yout[:], yout[:], beta_bc[:])
                nc.sync.dma_start(out[128 * qb:128 * qb + 128, :], yout[:])

            # ---------- schedule ----------
            # PE pstate warmup: ~3us of continuous dummy matmuls on a
            # memset tile (no DMA dependency) bring the tensor engine to
            # full clock before the first real projection.
            dmy = p_const.tile([128, 128], fp8, name="dmy", tag="dmy")
            nc.vector.memset(dmy[:], 1.0)
            warm = ps_kv.tile([128, 128], f32, name="warm", tag="kvp")
            for _ in range(30):
                nc.tensor.matmul(warm[:], dmy[:], dmy[:], start=True,
                                 stop=True, skip_group_check=True)

            k_proj(3)
            q_proj(3)
            v_proj(3)
            attn_row(3)
            k_proj(2)
            q_proj(2)
            v_proj(2)
            attn_row(2)
            k_proj(1)
            q_proj(1)
            v_proj(1)

            def post1():
                out_proj(2)
                ln_sums(2)
            attn_row(1, post=post1)
            k_proj(0)
            v_proj(0)

            # sv = sum of v over all kpos (for the fully-masked q = S-1 col)
            sv_ps = ps_kv.tile([1, 320], f32, name="sv_ps", tag="kvp")
            for kp in range(8):
                nc.tensor.matmul(sv_ps[:], ones8[:, :, 0:1], vp_sb[kp][:],
                                 perf_mode=DR,
                                 start=(kp == 0), stop=(kp == 7),
                                 skip_group_check=True)
            sv_row = p_const.tile([1, 320], f32, name="sv_row", tag="svr")
            nc.vector.tensor_copy(sv_row[:], sv_ps[:])
            # override ctx col S-1 with mean(v) per head
            for hi in range(HPC):
                t2f, halff = hi // 2, hi % 2
                pof = 64 * halff
                svc = ps_kv.tile([64, 1], f32, name="svc", tag="kvp")
                nc.tensor.matmul(svc[:], sv_row[0:1, 80 * hi:80 * hi + 64],
                                 one32[:], start=True, stop=True)
                nc.vector.tensor_scalar_mul(
                    ctx_sb[pof:pof + 64, t2f, S - 1:S], svc[:],
                    1.0 / float(S))
            q_proj(0)

            def post0():
                out_proj(3)
                ln_sums(3)
                out_proj(1)
                ln_sums(1)
            attn_row(0, post=post0)
            out_proj(0)
            # finals for 3,2,1 first (vars long ready): emitted BEFORE
            # ln_sums(0) so the in-order DVE queue doesn't park them behind
            # the qb0 rs/y round-trip wait. One Exp->Sqrt table switch.
            ln_final(3)
            ln_final(2)
            ln_final(1)
            ln_sums(0)
            ln_final(0)

    nc.compile()
    return nc


def _get_program(with_collective=True, ln_affine=False, qkv_bias=False):
    key = ("prog", with_collective, ln_affine, qkv_bias)
    if key not in _CACHE:
        _CACHE[key] = _build_program(with_collective, ln_affine, qkv_bias)
    return _CACHE[key]


def _pair_cols(w):
    """[1024, ncols] -> [128, 4, 2, ncols] d-block-pair layout, flattened
    to [128, 4*2*ncols] per-partition-contiguous."""
    n = w.shape[1]
    return np.ascontiguousarray(
        w.reshape(4, 2, 128, n).transpose(2, 0, 1, 3).reshape(128, 8 * n))


def _host_prep(x, Wq, bq, Wk, bk, Wv, bv, Wo, bo, gamma, beta):
    """Build the 8 per-core input dicts."""
    import ml_dtypes
    FP8 = ml_dtypes.float8_e4m3

    x = np.ascontiguousarray(np.asarray(x, np.float32))
    WqR = np.asarray(Wq, np.float32).reshape(D, H * DH)
    WkR = np.asarray(Wk, np.float32).reshape(D, H * DH)
    WvR = np.asarray(Wv, np.float32).reshape(D, H * DH)
    WoR = np.asarray(Wo, np.float32).reshape(H * DH, D)
    bqF = np.asarray(bq, np.float32).reshape(H * DH)
    bkF = np.asarray(bk, np.float32).reshape(H * DH)
    bvF = np.asarray(bv, np.float32).reshape(H * DH)
    boF = np.asarray(bo, np.float32).reshape(D)
    gF = np.asarray(gamma, np.float32).reshape(D)
    btF = np.asarray(beta, np.float32).reshape(D)

    # xT fp8: per window contiguous [128, 4, 2, 512]
    xt8 = []
    for b in range(B):
        xT = x[b].T.astype(FP8)  # [1024, 2048]
        t = (xT.reshape(4, 2, 128, NQB, 512)
             .transpose(3, 2, 0, 1, 4))  # [w, 128, kcp, 2, 512]
        xt8.append(np.ascontiguousarray(t))

    i = np.arange(128)[:, None]
    j = np.arange(128)[None, :]
    maskA = np.where(i >= j, MASKC, 0.0).astype(FP8)  # A[k,p]: k >= p
    eye = np.eye(128, dtype=np.float32)
    cwall = np.concatenate(
        [maskA.astype(np.float32), eye, eye, eye,
         np.ones((128, 32), np.float32),
         np.full((128, 128), MASKC, np.float32),
         np.ones((128, 128), np.float32)], axis=1).astype(FP8)

    biases_zero = not (bqF.any() or bkF.any() or bvF.any())

    in_maps = []
    for c in range(NCORES):
        b, hg = c // 4, c % 4
        cols = slice(256 * hg, 256 * hg + 256)
        wv_c = np.zeros((D, 320), np.float32)
        bv_c = np.zeros((1, 320), np.float32)
        for h2 in range(4):
            wv_c[:, 80 * h2:80 * h2 + 64] = WvR[:, 256 * hg + 64 * h2:
                                                256 * hg + 64 * h2 + 64]
            bv_c[0, 80 * h2:80 * h2 + 64] = bvF[256 * hg + 64 * h2:
                                                256 * hg + 64 * h2 + 64]
        wo_c = np.ascontiguousarray(
            WoR[cols, :].reshape(2, 128, 1024).transpose(1, 0, 2)
            .reshape(128, 2048))
        m = {
            "xt8": xt8[b],
            "wk8": _pair_cols(WkR[:, cols]).astype(FP8),
            "wq8": _pair_cols(WqR[:, cols]).astype(FP8),
            "wv8": _pair_cols(wv_c).astype(FP8),
            "wo8": wo_c.astype(FP8),
            "cwall": cwall,
            "xres": boF[None, :] + np.concatenate(
                [x[b, QB * j2 + 128 * hg:QB * j2 + 128 * hg + 128]
                 for j2 in range(NQB)], axis=0),
        }
        if not biases_zero:
            m["onesr32"] = np.ones((1, 512), np.float32)
            m["bq_r"] = bqF[cols].reshape(1, 256).copy()
            m["bk_r"] = bkF[cols].reshape(1, 256).copy()
            m["bv_r"] = bv_c
        m["gamma_r"] = gF[None, :].copy()
        m["beta_r"] = btF[None, :].copy()
        in_maps.append(m)
    return in_maps


def kernel(**inputs):
    from concourse.bass_utils import run_bass_kernel_spmd

    gamma = np.asarray(inputs["gamma"], np.float32)
    beta = np.asarray(inputs["beta"], np.float32)
    ln_affine = not (np.all(gamma == 1.0) and np.all(beta == 0.0))
    qkv_bias = any(np.asarray(inputs[k], np.float32).any()
                   for k in ("bq", "bk", "bv"))
    nc = _get_program(with_collective=True, ln_affine=ln_affine,
                      qkv_bias=qkv_bias)
    in_maps = _host_prep(**inputs)
    drop = []
    if not ln_affine:
        drop += ["gamma_r", "beta_r"]
    for m in in_maps:
        for k in drop:
            m.pop(k, None)
    res = run_bass_kernel_spmd(nc, in_maps, list(range(NCORES)))
    full = np.empty((B, S, D), np.float32)
    for c in range(NCORES):
        b, r = c // 4, c % 4
        o = res.results[c]["out"]
        for j in range(NQB):
            full[b, QB * j + 128 * r:QB * j + 128 * r + 128, :] = \
                o[128 * j:128 * j + 128]
    return full


# revision 75
# speedup vs baseline: 1.0072x; 1.0072x over previous
"""Trainium2 Bass kernel for causal (strict-future-masked) MHA + residual + LayerNorm.

Reference semantics (Keras MultiHeadAttention, inference):
    q,k,v = einsum(x, W{q,k,v}) + b    [B,S,H,DH]
    scores = q.k / sqrt(DH); mask allows j > i (STRICT UPPER triangle);
    masked entries get -1e9 added (in fp32 this makes the fully-masked row
    S-1 collapse to exactly -1e9 -> uniform softmax = 1/S).
    ctx = probs @ v; out = ctx @ Wo + bo; y = LN(x + out) * gamma + beta.

Shapes: B=2, S=2048, D=1024, H=16, DH=64.

Sharding (8 cores): core c -> batch b = c//4, head-group hg = c%4 (4 heads),
RS rank r = c%4. Each core computes q/k/v + attention + out-proj partial for
its 4 heads over the full sequence, ReduceScatter([2048,1024] bf16) within
its 4-core batch group yields rows [512r, 512r+512) of the head-summed
attn_out, then residual + LayerNorm locally. Host assembles 8 x [512,1024].

Device-side design (v2 -- exp-stream-critical schedule):
  The scalar (Activation) engine's exp stream (~69K cols @ 0.83ns/col) is
  the critical resource; everything else is organized to hide under it.
  - QKV via fp8e4 DoubleRow matmuls (x and W quantized fp8 on host, pairs
    of 128-row d-blocks per instruction at 0.5 cyc/col).
  - qT/kT stored bf16 [128 = 2 heads x 64dh, S]; v stored fp8 in even/odd
    kb-pair tiles [128, 2, 4x(64+onescol)] (ones col -> Z row via matmul).
  - Rows processed qb = 3,2,1,0 (strict-upper mask: row qb needs only KV
    windows >= qb) interleaved with JIT KV/Q projections; the exp stream
    starts a few us in and everything else hides under it.
  - Causal masking: -240 band add-matmuls on the PE into score PSUM
    (lhsT = lower-tri(-240), rhs = I / [I|I]); exp underflows to exact 0
    in fp8. No vector-engine masking.
  - E tiles fp8e4 (exp bias -ln8 keeps E <= ~40 << 240-max); full kb
    blocks pair into [128, 2, 512] tiles; the rr3/rr2 diagonals pair too
    (rr2 zero-padded via one memset) -> ctx mostly fp8 DoubleRow.
  - Out-proj fp8 DoubleRow over t2-pairs; stage -> DRAM bf16; chunked
    ReduceScatter per q-block overlaps the remaining attention.
  - DMAs are merged into few large transfers (one per xt window, one
    weight wall, one const wall, one xres, one stage per qb, one out) and
    all pure loads issue up front: the cost model charges ~625ns of a
    shared HWDGE port per DMA and the in-order queues otherwise propagate
    store-waits into load stalls.
  - LayerNorm: sums/var on DVE mid-stream; Sqrt lives in a different
    activation table than Exp, so the 4 sqrt+scale finals are deferred
    past the last exp (single table switch).
"""

import numpy as np

B, S, D, H, DH = 2, 2048, 1024, 16, 64
HPC = 4            # heads per core
NCORES = 8
QB = 512           # q-block
NQB = S // QB      # 4
KBLK = 128         # kpos block
NKB = S // KBLK    # 16
SCALE = 1.0 / 8.0  # 1/sqrt(DH)
LN8 = 2.0794415416798357  # ln(8): exp bias; E = exp(s/8 - ln8) <= ~40
EPS = 1.0e-6
MASKC = -240.0     # band-add constant; exp((s-240)/8 - ln8) -> fp8 0
# Schraudolph fast-exp (DVE/Pool offload): int32(SA*s + SB) bitcast to f32
# approximates exp(s/8 - ln8) with ~2% rms error; the Z normalization is
# computed from the same E so the bias largely cancels in the softmax.
SEXP_A = 12102203.161561485 / 8.0
SEXP_B = float(127 * (1 << 23) - 486411 - 3 * (1 << 23))
# full-block pairs (per head, by index in the pair list) offloaded to DVE
SEXP_DVE = {3: (), 2: (), 1: (), 0: ()}
SEXP_C_DVE = False  # C diagonal units' fast-exp on DVE

_CACHE = {}


def _build_program(with_collective=True, ln_affine=False, qkv_bias=False):
    """Build + compile the SPMD Bass program (identical on all 8 cores)."""
    import concourse.bass as bass
    import concourse.tile as tile
    from concourse import bacc, mybir

    f32 = mybir.dt.float32
    f32r = mybir.dt.float32r
    i32 = mybir.dt.int32
    bf16 = mybir.dt.bfloat16
    fp8 = mybir.dt.float8e4
    Alu = mybir.AluOpType
    Act = mybir.ActivationFunctionType
    DR = mybir.MatmulPerfMode.DoubleRow

    nc = bacc.Bacc("TRN2", target_bir_lowering=False, debug=False,
                   num_devices=NCORES)

    # ---- external I/O ----
    # xt8: per window one contiguous [128, 4kcp, 2, 512] fp8 block
    xt8 = nc.dram_tensor("xt8", [NQB, 128, 4, 2, 512], fp8,
                         kind="ExternalInput").ap()
    # weights as separate tensors, DMA'd in criticality order:
    # wk before xt3 (first K proj), wq right after (first Q proj), then wv,
    # wo much later (first needed by out_proj(2))
    wk_d = nc.dram_tensor("wk8", [128, 2048], fp8, kind="ExternalInput").ap()
    wq_d = nc.dram_tensor("wq8", [128, 2048], fp8, kind="ExternalInput").ap()
    wv_d = nc.dram_tensor("wv8", [128, 2560], fp8, kind="ExternalInput").ap()
    wo_d = nc.dram_tensor("wo8", [128, 2048], fp8, kind="ExternalInput").ap()
    # const wall: maskA [128,128] | maskI [128,128] | maskII [128,256]
    # | ones8 [128,2] -> [128, 514] fp8
    cwall = nc.dram_tensor("cwall", [128, 800], fp8,
                           kind="ExternalInput").ap()
    xres = nc.dram_tensor("xres", [QB, D], f32, kind="ExternalInput").ap()
    if qkv_bias:
        onesr_d = nc.dram_tensor("onesr32", [1, 512], f32,
                                 kind="ExternalInput").ap()
        bq_d = nc.dram_tensor("bq_r", [1, 256], f32, kind="ExternalInput").ap()
        bk_d = nc.dram_tensor("bk_r", [1, 256], f32, kind="ExternalInput").ap()
        bv_d = nc.dram_tensor("bv_r", [1, 320], f32, kind="ExternalInput").ap()
    if ln_affine:
        gamma_r = nc.dram_tensor("gamma_r", [1, D], f32,
                                 kind="ExternalInput").ap()
        beta_r = nc.dram_tensor("beta_r", [1, D], f32,
                                kind="ExternalInput").ap()
    out = nc.dram_tensor("out", [QB, D], f32, kind="ExternalOutput").ap()

    # internal DRAM for the chunked collectives (one per q-block)
    attn_dram_l = [nc.dram_tensor(f"attn_dram{j}", [QB, D], bf16)
                   for j in range(NQB)]
    rs_dram_l = [nc.dram_tensor(f"rs_dram{j}", [128, D], bf16)
                 for j in range(NQB)]

    ROWS = [3, 2, 1, 0]

    with tile.TileContext(nc) as tc, \
         nc.allow_low_precision(reason="fp8/bf16 attention path"):
        from contextlib import ExitStack
        with ExitStack() as ctx:
            # ---------- persistent pools ----------
            p_const = ctx.enter_context(tc.tile_pool(name="const", bufs=1))
            p_w = ctx.enter_context(tc.tile_pool(name="w", bufs=1))
            p_qk = ctx.enter_context(tc.tile_pool(name="qk", bufs=1))
            p_v = ctx.enter_context(tc.tile_pool(name="v", bufs=1))
            p_ctx = ctx.enter_context(tc.tile_pool(name="ctxp", bufs=1))
            p_xt = ctx.enter_context(tc.tile_pool(name="xt", bufs=1))
            p_e = ctx.enter_context(tc.tile_pool(name="e", bufs=8))
            p_es = ctx.enter_context(tc.tile_pool(name="es", bufs=4))
            p_z = ctx.enter_context(tc.tile_pool(name="z", bufs=4))
            p_bcn = ctx.enter_context(tc.tile_pool(name="bcn", bufs=3))
            p_stage = ctx.enter_context(tc.tile_pool(name="stage", bufs=2))
            p_lnp = ctx.enter_context(tc.tile_pool(name="lnp", bufs=1))
            p_lns = ctx.enter_context(tc.tile_pool(name="lns", bufs=2))
            p_lnc = ctx.enter_context(tc.tile_pool(name="lnc", bufs=10))
            ps_kv = ctx.enter_context(
                tc.tile_pool(name="ps_kv", bufs=2, space="PSUM"))
            ps_s = ctx.enter_context(
                tc.tile_pool(name="ps_s", bufs=2, space="PSUM"))
            ps_cu = ctx.enter_context(
                tc.tile_pool(name="ps_cu", bufs=2, space="PSUM"))

            # ---------- merged loads, all issued up front (SP queue) -------
            wkt = p_w.tile([128, 2048], fp8, name="wkt", tag="wkt")
            nc.sync.dma_start(wkt[:], wk_d[:])
            wk_sb = wkt.rearrange("p (k t m) -> p k t m", k=4, t=2)

            xt_sb = {}
            t = p_xt.tile([128, 4, 2, 512], fp8, name="xt3", tag="xt3")
            nc.sync.dma_start(t[:], xt8[3])
            xt_sb[3] = t

            wqt = p_w.tile([128, 2048], fp8, name="wqt", tag="wqt")
            nc.sync.dma_start(wqt[:], wq_d[:])
            wq_sb = wqt.rearrange("p (k t m) -> p k t m", k=4, t=2)

            # masks are first needed by the band adds at ~8us; the warmup
            # runs on a memset tile, so this load can follow the projections
            cw = p_const.tile([128, 800], fp8, name="cw", tag="cw")
            nc.sync.dma_start(cw[:], cwall[:])
            maskA = cw[:, 0:128]
            maskI = cw[:, 128:256]
            maskII = cw[:, 256:512]
            ones8 = cw[:, 512:544].rearrange("p (t o) -> p t o", o=16)
            negc_row = cw[0:1, 544:672]
            ones_row = cw[0:1, 672:800]

            wvt = p_w.tile([128, 2560], fp8, name="wvt", tag="wvt")
            nc.sync.dma_start(wvt[:], wv_d[:])
            wv_sb = wvt.rearrange("p (k t m) -> p k t m", k=4, t=2)

            for w in (2, 1, 0):
                t = p_xt.tile([128, 4, 2, 512], fp8, name=f"xt{w}",
                              tag=f"xt{w}")
                nc.sync.dma_start(t[:], xt8[w])
                xt_sb[w] = t

            wot = p_w.tile([128, 2048], fp8, name="wot", tag="wot")
            nc.sync.dma_start(wot[:], wo_d[:])
            wo_ap = wot.rearrange("p (t m) -> p t m", t=2)

            # xres: one DMA into [128, 4, 1024] (row chunk qb on axis 1)
            xr_all = p_lnp.tile([128, 4, D], f32, name="xr_all", tag="xr")
            nc.sync.dma_start(
                xr_all[:], xres.rearrange("(q p) d -> p q d", p=128))

            one32 = p_const.tile([1, 1], f32, name="one32", tag="one32")
            nc.vector.memset(one32[:], 1.0)
            eps_col = p_const.tile([128, 1], f32, name="eps_col", tag="eps")
            nc.vector.memset(eps_col[:], EPS)
            ln8_col = p_const.tile([128, 1], f32, name="ln8_col", tag="ln8")
            nc.vector.memset(ln8_col[:], -LN8)

            if qkv_bias:
                onesr = p_const.tile([1, 512], f32, name="onesr", tag="onesr")
                nc.sync.dma_start(onesr[:], onesr_d[:])
                bq_row = p_const.tile([1, 256], f32, name="bq_row", tag="bqr")
                nc.sync.dma_start(bq_row[:], bq_d[:])
                bk_row = p_const.tile([1, 256], f32, name="bk_row", tag="bkr")
                nc.sync.dma_start(bk_row[:], bk_d[:])
                bv_row = p_const.tile([1, 320], f32, name="bv_row", tag="bvr")
                nc.sync.dma_start(bv_row[:], bv_d[:])
            if ln_affine:
                gamma_row = p_const.tile([1, D], f32, name="gamma_row",
                                         tag="gr")
                nc.sync.dma_start(gamma_row[:], gamma_r[:])
                beta_row = p_const.tile([1, D], f32, name="beta_row",
                                        tag="br")
                nc.sync.dma_start(beta_row[:], beta_r[:])

            # persistent activations
            qT_sb = [p_qk.tile([128, S], bf16, name=f"qT{t2}", tag=f"qT{t2}")
                     for t2 in range(2)]
            kT_sb = [p_qk.tile([128, S], bf16, name=f"kT{t2}", tag=f"kT{t2}")
                     for t2 in range(2)]
            # v pair tiles: kp pairs (2kp, 2kp+1); cols 65*hi..65*hi+64 + ones
            # head stride 80 = 5x16B: dual-fp8 Ldweights requires outer AP
            # steps 16B-aligned (cols 80h..80h+63 = weights, 80h+64 = ones)
            vp_sb = [p_v.tile([128, 2, 320], fp8, name=f"vp{kp}",
                              tag=f"vp{kp}") for kp in range(8)]
            ctx_sb = p_ctx.tile([128, 2, S], fp8, name="ctxT", tag="ctxT")

            # LN persistent tiles (finals deferred past the last exp)
            y32_sb = {qb: p_lnp.tile([128, D], f32, name=f"y32_{qb}",
                                     tag=f"y32_{qb}") for qb in ROWS}
            varc_sb = {qb: p_lnp.tile([128, 1], f32, name=f"varc{qb}",
                                      tag=f"varc{qb}") for qb in ROWS}
            negmu_sb = {qb: p_lnp.tile([128, 1], f32, name=f"negmu{qb}",
                                       tag=f"negmu{qb}") for qb in ROWS}

            if ln_affine:
                gamma_bc = p_const.tile([128, D], f32, name="gamma_bc",
                                        tag="gbc")
                nc.gpsimd.partition_broadcast(gamma_bc[:], gamma_row[:])
                beta_bc = p_const.tile([128, D], f32, name="beta_bc",
                                       tag="bbc")
                nc.gpsimd.partition_broadcast(beta_bc[:], beta_row[:])

            def k_proj(w):
                xt_w = xt_sb[w]
                # K for window w -> kT bf16 (copies on Pool engine)
                for t2 in range(2):
                    acc = ps_kv.tile([128, 512], f32, name="kvp", tag="kvp")
                    for kcp in range(4):
                        nc.tensor.matmul(
                            acc[:],
                            wk_sb[:, kcp, :, 128 * t2:128 * t2 + 128],
                            xt_w[:, kcp, :, :], perf_mode=DR,
                            start=(kcp == 0),
                            stop=(kcp == 3 and not qkv_bias))
                    if qkv_bias:
                        nc.tensor.matmul(
                            acc[:], bk_row[0:1, 128 * t2:128 * t2 + 128],
                            onesr[:], start=False, stop=True)
                    # DVE: the gpsimd library load blocks the Pool queue
                    # for ~7us at startup, and Pool copies are slower
                    nc.vector.tensor_copy(
                        kT_sb[t2][:, 512 * w:512 * w + 512], acc[:])

            def v_proj(w):
                xt_w = xt_sb[w]
                # V for window w -> fp8 pair tiles (copies on DVE)
                for tsub in range(4):
                    kb = 4 * w + tsub
                    kp, half = kb // 2, kb % 2
                    acc = ps_kv.tile([128, 320], f32, name="vpp", tag="kvp")
                    for kcp in range(4):
                        nc.tensor.matmul(
                            acc[:],
                            xt_w[:, kcp, :, 128 * tsub:128 * tsub + 128],
                            wv_sb[:, kcp, :, :], perf_mode=DR,
                            start=(kcp == 0),
                            stop=(kcp == 3 and not qkv_bias))
                    if qkv_bias:
                        nc.tensor.matmul(
                            acc[:], onesr[0:1, 0:128], bv_row[:],
                            start=False, stop=True)
                    nc.vector.tensor_copy(vp_sb[kp][:, half, :], acc[:])
                # ones cols for the Z-row trick (after both halves land)
                for kp in (2 * w, 2 * w + 1):
                    vcols = vp_sb[kp].rearrange("p t (h e) -> p t h e", e=80)
                    nc.vector.memset(vcols[:, :, :, 64:65], 1.0)

            def q_proj(qb):
                xt_w = xt_sb[qb]
                for t2 in range(2):
                    acc = ps_kv.tile([128, 512], f32, name="qp", tag="kvp")
                    for kcp in range(4):
                        nc.tensor.matmul(
                            acc[:],
                            wq_sb[:, kcp, :, 128 * t2:128 * t2 + 128],
                            xt_w[:, kcp, :, :], perf_mode=DR,
                            start=(kcp == 0),
                            stop=(kcp == 3 and not qkv_bias))
                    if qkv_bias:
                        nc.tensor.matmul(
                            acc[:], bq_row[0:1, 128 * t2:128 * t2 + 128],
                            onesr[:], start=False, stop=True)
                    nc.vector.tensor_copy(
                        qT_sb[t2][:, 512 * qb:512 * qb + 512], acc[:])

            def score_mm(sT_roi, t2, po, kb, qoff, w, start, stop):
                nc.tensor.matmul(
                    sT_roi,
                    kT_sb[t2][po:po + 64, 128 * kb:128 * kb + 128],
                    qT_sb[t2][po:po + 64, qoff:qoff + w],
                    start=start, stop=stop, skip_group_check=True)

            def band_add(sT_roi, rhs, stop):
                nc.tensor.matmul(sT_roi, maskA, rhs,
                                 start=False, stop=stop,
                                 skip_group_check=True)

            def exp_to(e_roi, s_roi):
                nc.scalar.activation(e_roi, s_roi, Act.Exp,
                                     scale=SCALE, bias=ln8_col[:])

            def attn_row(qb, post=None):
                # ctx matmuls are emitted one unit LATE: the in-order PE
                # queue then issues the next unit's score matmuls while the
                # current unit's exps run, keeping the Act stream gapless.
                qoff = QB * qb
                kb0 = 4 * qb
                pend = [None]

                def emit(scores_fn, ctx_fn):
                    scores_fn()
                    if pend[0] is not None:
                        pend[0]()
                    pend[0] = ctx_fn

                for t2 in range(2):
                    for half in range(2):
                        po = 64 * half
                        hi = 2 * t2 + half
                        ctxu = ps_cu.tile([65, QB], f32, name="ctxu",
                                          tag="cu")
                        vsl = slice(80 * hi, 80 * hi + 65)
                        first = [True]

                        def mk_pair(kp, ctxu=ctxu, t2=t2, po=po, vsl=vsl,
                                    first=first):
                            ep = p_e.tile([128, 2, 512], fp8, name="ep",
                                          tag="ep")

                            def scores():
                                sT2 = ps_s.tile([128, 2, QB], f32,
                                                name="sT2", tag="sT")
                                for j in range(2):
                                    score_mm(sT2[:, j, :], t2, po,
                                             2 * kp + j, qoff, QB,
                                             True, True)
                                exp_to(ep[:], sT2[:])

                            def ctx():
                                nc.tensor.matmul(
                                    ctxu[:], vp_sb[kp][:, :, vsl], ep[:],
                                    perf_mode=DR, start=first[0],
                                    stop=False, skip_group_check=True)
                                first[0] = False
                            return scores, ctx

                        def mk_pair_dve(kp, ctxu=ctxu, t2=t2, po=po,
                                        vsl=vsl, first=first):
                            # Schraudolph fast-exp on the DVE; E stays f32
                            # bits (bitcast f32r), ctx via two singles
                            es = p_es.tile([128, 2, 512], i32, name="es",
                                           tag="es")

                            def scores():
                                for j in range(2):
                                    sT = ps_s.tile([128, QB], f32,
                                                   name="sT", tag="sT")
                                    score_mm(sT[:], t2, po, 2 * kp + j,
                                             qoff, QB, True, True)
                                    nc.vector.tensor_scalar(
                                        es[:, j, :], sT[:], SEXP_A, SEXP_B,
                                        Alu.mult, Alu.add)

                            def ctx():
                                for j in range(2):
                                    nc.tensor.matmul(
                                        ctxu[:], vp_sb[kp][:, j, vsl],
                                        es[:, j, :].bitcast(f32r),
                                        start=(first[0] and j == 0),
                                        stop=False, skip_group_check=True)
                                first[0] = False
                            return scores, ctx

                        def mk_ab(ctxu=ctxu, t2=t2, po=po, vsl=vsl,
                                  first=first):
                            # A = rr2 @ [0,384) zero-padded to 512 (memset),
                            # B = rr3 @ [0,512); v pair (rr2, rr3) =
                            # vp[2qb+1] halves (0, 1) in natural order.
                            eab = p_e.tile([128, 2, 512], fp8, name="eab",
                                           tag="ep")

                            def scores():
                                sT2 = ps_s.tile([128, 2, QB], f32,
                                                name="sT2", tag="sT")
                                score_mm(sT2[:, 0, 0:384], t2, po, kb0 + 2,
                                         qoff, 384, True, False)
                                band_add(sT2[:, 0, 256:384], maskI, False)
                                # fill the rr2 pad [384,512) with -240 (on
                                # the zeroed bank) so one merged exp covers
                                # both banks and lands exact fp8 zeros there
                                nc.tensor.matmul(
                                    sT2[:, 0, 384:512], negc_row, ones_row,
                                    start=False, stop=True,
                                    skip_group_check=True)
                                score_mm(sT2[:, 1, :], t2, po, kb0 + 3,
                                         qoff, QB, True, False)
                                band_add(sT2[:, 1, 384:512], maskI, True)
                                exp_to(eab[:], sT2[:])

                            def ctx():
                                nc.tensor.matmul(
                                    ctxu[:], vp_sb[2 * qb + 1][:, :, vsl],
                                    eab[:], perf_mode=DR, start=first[0],
                                    stop=False, skip_group_check=True)
                                first[0] = False
                            return scores, ctx

                        def mk_c(ctxu=ctxu, t2=t2, po=po, vsl=vsl):
                            # C: rr1 @ [0,256) + rr0 @ [256,384); bands via
                            # one [I|I] add at [128,384). Optionally
                            # Schraudolph on GpSimd (masked entries land at
                            # ~1e-14 instead of exact 0 -- negligible).
                            if SEXP_C_DVE:
                                eC = p_es.tile([128, 384], i32, name="eC",
                                               tag="es")
                            else:
                                eC = p_e.tile([128, 384], fp8, name="eC",
                                              tag="ep")

                            def scores():
                                sTw = ps_s.tile([128, 2, QB], f32,
                                                name="sTc", tag="sT")
                                sT = sTw[:, 0, 0:384]
                                score_mm(sT[:, 0:256], t2, po, kb0 + 1,
                                         qoff, 256, True, False)
                                score_mm(sT[:, 256:384], t2, po, kb0,
                                         qoff, 128, False, False)
                                band_add(sT[:, 128:384], maskII, True)
                                if SEXP_C_DVE:
                                    nc.vector.tensor_scalar(
                                        eC[:], sT[:], SEXP_A, SEXP_B,
                                        Alu.mult, Alu.add)
                                else:
                                    exp_to(eC[:], sT[:])

                            def ctx():
                                e1 = eC[:, 0:256]
                                e0 = eC[:, 256:384]
                                if SEXP_C_DVE:
                                    e1 = e1.bitcast(f32r)
                                    e0 = e0.bitcast(f32r)
                                nc.tensor.matmul(
                                    ctxu[:, 0:256], vp_sb[2 * qb][:, 1, vsl],
                                    e1, start=False, stop=False,
                                    skip_group_check=True)
                                nc.tensor.matmul(
                                    ctxu[:, 0:128], vp_sb[2 * qb][:, 0, vsl],
                                    e0, start=False, stop=True,
                                    skip_group_check=True)
                            return scores, ctx

                        for pi, kp in enumerate(range(2 * qb + 2, 8)):
                            if pi in SEXP_DVE[qb]:
                                emit(*mk_pair_dve(kp))
                            else:
                                emit(*mk_pair(kp))
                        emit(*mk_ab())
                        emit(*mk_c())

                        def norm(ctxu=ctxu, t2=t2, po=po, qoff=qoff,
                                 half=half):
                            # ctx = ctxu[0:64] / Z  (Z = row 64). The very
                            # last head of the last row is normalized in
                            # 128-col chunks so out_proj qt0 (the LN-chain
                            # gate) starts ~1.5us earlier.
                            if qb == 0 and t2 == 1 and half == 1:
                                for cc in range(4):
                                    cs = slice(128 * cc, 128 * cc + 128)
                                    zinv = p_z.tile([1, 128], f32,
                                                    name="zinv", tag="zinv")
                                    nc.vector.reciprocal(
                                        zinv[:], ctxu[64:65, cs])
                                    zbs = p_bcn.tile([64, 128], f32,
                                                     name="zbs", tag="zbs")
                                    nc.gpsimd.partition_broadcast(
                                        zbs[:], zinv[:])
                                    nc.vector.tensor_mul(
                                        ctx_sb[po:po + 64, t2,
                                               qoff + 128 * cc:
                                               qoff + 128 * cc + 128],
                                        ctxu[0:64, cs], zbs[:])
                                return
                            zinv = p_z.tile([1, QB], f32, name="zinv",
                                            tag="zinv")
                            nc.vector.reciprocal(zinv[:], ctxu[64:65, :])
                            zbs = p_bcn.tile([64, QB], f32, name="zbs",
                                             tag="zbs")
                            nc.gpsimd.partition_broadcast(zbs[:], zinv[:])
                            nc.vector.tensor_mul(
                                ctx_sb[po:po + 64, t2, qoff:qoff + QB],
                                ctxu[0:64, :], zbs[:])

                        prev_ctx = pend[0]

                        def flush_and_norm(prev_ctx=prev_ctx, norm=norm):
                            prev_ctx()
                            norm()
                        pend[0] = flush_and_norm
                        if post is not None and t2 == 0 and half == 0:
                            # deferred work from the previous row: its PE
                            # burst lands inside this row's exp stream
                            # (head 0 is already buffered ahead)
                            post()
                            post = None
                if pend[0] is not None:
                    pend[0]()

            def out_proj(qb):
                # qt0 gets its own tile + DMA with plain 2D access patterns:
                # the LN chain (and the timed-path y read) depends only on
                # rows 0:128, so this slice must move with minimum latency.
                stage0 = p_stage.tile([128, D], bf16, name="stage0",
                                      tag="stage0")
                stage = p_stage.tile([128, 3, D], bf16, name="stage",
                                     tag="stage")
                ad = attn_dram_l[qb].rearrange("(t p) d -> p t d", p=128)
                for qtl in range(4):
                    toff = QB * qb + 128 * qtl
                    for ob in range(2):
                        acc = ps_cu.tile([128, 512], f32, name="op", tag="cu")
                        nc.tensor.matmul(
                            acc[:], ctx_sb[:, :, toff:toff + 128],
                            wo_ap[:, :, 512 * ob:512 * ob + 512],
                            perf_mode=DR, start=True, stop=True)
                        # PSUM readers are DVE/Act only. For qb0 the Act
                        # engine is idle (exp stream over) and Copy is in
                        # the exp act-table: split copies Act/DVE there.
                        dst = (stage0[:, 512 * ob:512 * ob + 512] if qtl == 0
                               else stage[:, qtl - 1,
                                          512 * ob:512 * ob + 512])
                        if qb == 0 and ob == 1:
                            nc.scalar.copy(dst, acc[:])
                        else:
                            nc.vector.tensor_copy(dst, acc[:])
                    if qtl == 0:
                        nc.sync.dma_start(attn_dram_l[qb][0:128, :],
                                          stage0[:])
                nc.sync.dma_start(ad[:, 1:4, :], stage[:])
                if with_collective:
                    nc.gpsimd.collective_compute(
                        "ReduceScatter",
                        mybir.AluOpType.add,
                        replica_groups=[[0, 1, 2, 3], [4, 5, 6, 7]],
                        ins=[attn_dram_l[qb][:]],
                        outs=[rs_dram_l[qb][:]],
                    )


            def ln_sums(qb):
                # residual add + sum/sq-sum (DVE); var = sqsum/D - mu^2 via
                # column algebra; sqrt/scale deferred past the last exp
                yb = p_lns.tile([128, D], bf16, name="yb", tag="yb")
                # real path: read the collective's output; timed path: read
                # the local partial directly (the RS itself is accounted by
                # the fixed tail constant, not the cost model)
                ysrc = (rs_dram_l[qb][:] if with_collective
                        else attn_dram_l[qb][0:128, :])
                nc.gpsimd.dma_start(yb[:], ysrc)
                y32 = y32_sb[qb]
                # TensorScalarPtr-class ops (stt with tensor operand) are
                # DVE-only on real hardware
                eng = nc.vector
                ysum = p_lnc.tile([128, 1], f32, name="ysum", tag="lncol")
                eng.scalar_tensor_tensor(
                    y32[:], yb[:], 1.0, xr_all[:, qb, :], Alu.mult, Alu.add,
                    accum_out=ysum[:])
                negmu = negmu_sb[qb]
                nc.vector.tensor_scalar_mul(negmu[:], ysum[:],
                                            -1.0 / float(D))
                sq = p_lns.tile([128, D], f32, name="sq", tag="sq")
                sqsum = p_lnc.tile([128, 1], f32, name="sqsum", tag="lncol")
                eng.scalar_tensor_tensor(
                    sq[:], y32[:], 1.0, y32[:], Alu.mult, Alu.mult,
                    accum_out=sqsum[:])
                musq = p_lnc.tile([128, 1], f32, name="musq", tag="lncol")
                nc.vector.tensor_tensor(musq[:], negmu[:], negmu[:],
                                        Alu.mult)
                # AP-scalar (Ptr) tensor_scalar is DVE-only on real hw
                nc.vector.tensor_scalar(varc_sb[qb][:], sqsum[:],
                                        1.0 / float(D), musq[:],
                                        Alu.mult, Alu.subtract)

            def ln_final(qb):
                # rstd = rsqrt(var + eps) fully on the DVE (Quake-style
                # bit-trick init + 2 Newton steps on [128,1] columns, ~1e-6
                # rel err). Keeps the Activation engine exp-only: the tile
                # scheduler otherwise interleaves Sqrt activations into the
                # exp stream at 2x1283ns act-table loads apiece.
                ve = p_lnc.tile([128, 1], f32, name="ve", tag="lncol")
                nc.vector.tensor_scalar_add(ve[:], varc_sb[qb][:], EPS)
                i1 = p_lnc.tile([128, 1], i32, name="i1", tag="lncol")
                nc.vector.tensor_scalar(i1[:], ve.bitcast(i32)[:], 1,
                                        -1, Alu.logical_shift_right,
                                        Alu.bitwise_xor)
                u0 = p_lnc.tile([128, 1], f32, name="u0", tag="lncol")
                nc.vector.tensor_scalar_add(u0.bitcast(i32)[:], i1[:],
                                            0x5F3759E0)
                u = u0
                for it in range(2):
                    a = p_lnc.tile([128, 1], f32, name="nta", tag="lncol")
                    nc.vector.tensor_tensor(a[:], u[:], u[:], Alu.mult)
                    b = p_lnc.tile([128, 1], f32, name="ntb", tag="lncol")
                    nc.vector.tensor_tensor(b[:], a[:], ve[:], Alu.mult)
                    c = p_lnc.tile([128, 1], f32, name="ntc", tag="lncol")
                    nc.vector.tensor_scalar(c[:], b[:], -0.5, 1.5,
                                            Alu.mult, Alu.add)
                    un = p_lnc.tile([128, 1], f32, name="ntu", tag="lncol")
                    nc.vector.tensor_tensor(un[:], u[:], c[:], Alu.mult)
                    u = un
                rstd = u
                mr = p_lnc.tile([128, 1], f32, name="mr", tag="lncol")
                nc.vector.tensor_tensor(mr[:], negmu_sb[qb][:], rstd[:],
                                        Alu.mult)
                yout = p_lns.tile([128, D], f32, name="yout", tag="yout")
                # AP-scalar (Ptr) tensor_scalar is DVE-only on real hw
                nc.vector.tensor_scalar(yout[:], y32_sb[qb][:], rstd[:],
                                        mr[:], Alu.mult, Alu.add)
                if ln_affine:
                    nc.vector.scalar_tensor_tensor(
                        yout[:], yout[:], 1.0, gamma_bc[:],
                        Alu.mult, Alu.mult)
                    nc.vector.tensor_add(yout[:], yout[:], beta_bc[:])
                nc.sync.dma_start(out[128 * qb:128 * qb + 128, :], yout[:])

            # ---------- schedule ----------
            # PE pstate warmup: ~3us of continuous dummy matmuls on a
            # memset tile (no DMA dependency) bring the tensor engine to
            # full clock before the first real projection.
            dmy = p_const.tile([128, 128], fp8, name="dmy", tag="dmy")
            nc.vector.memset(dmy[:], 1.0)
            warm = ps_kv.tile([128, 128], f32, name="warm", tag="kvp")
            for _ in range(30):
                nc.tensor.matmul(warm[:], dmy[:], dmy[:], start=True,
                                 stop=True, skip_group_check=True)

            k_proj(3)
            q_proj(3)
            v_proj(3)
            attn_row(3)
            k_proj(2)
            q_proj(2)
            v_proj(2)
            attn_row(2)
            k_proj(1)
            q_proj(1)
            v_proj(1)

            def post1():
                out_proj(2)
                ln_sums(2)
            attn_row(1, post=post1)
            k_proj(0)
            v_proj(0)

            # sv = sum of v over all kpos (for the fully-masked q = S-1 col)
            sv_ps = ps_kv.tile([1, 320], f32, name="sv_ps", tag="kvp")
            for kp in range(8):
                nc.tensor.matmul(sv_ps[:], ones8[:, :, 0:1], vp_sb[kp][:],
                                 perf_mode=DR,
                                 start=(kp == 0), stop=(kp == 7),
                                 skip_group_check=True)
            sv_row = p_const.tile([1, 320], f32, name="sv_row", tag="svr")
            nc.vector.tensor_copy(sv_row[:], sv_ps[:])
            # override ctx col S-1 with mean(v) per head
            for hi in range(HPC):
                t2f, halff = hi // 2, hi % 2
                pof = 64 * halff
                svc = ps_kv.tile([64, 1], f32, name="svc", tag="kvp")
                nc.tensor.matmul(svc[:], sv_row[0:1, 80 * hi:80 * hi + 64],
                                 one32[:], start=True, stop=True)
                nc.vector.tensor_scalar_mul(
                    ctx_sb[pof:pof + 64, t2f, S - 1:S], svc[:],
                    1.0 / float(S))
            q_proj(0)

            def post0():
                out_proj(3)
                ln_sums(3)
                out_proj(1)
                ln_sums(1)
            attn_row(0, post=post0)
            out_proj(0)
            # finals for 3,2,1 first (vars long ready): emitted BEFORE
            # ln_sums(0) so the in-order DVE queue doesn't park them behind
            # the qb0 rs/y round-trip wait. One Exp->Sqrt table switch.
            ln_final(3)
            ln_final(2)
            ln_final(1)
            ln_sums(0)
            ln_final(0)

    nc.compile()
    return nc


def _get_program(with_collective=True, ln_affine=False, qkv_bias=False):
    key = ("prog", with_collective, ln_affine, qkv_bias)
    if key not in _CACHE:
        _CACHE[key] = _build_program(with_collective, ln_affine, qkv_bias)
    return _CACHE[key]


def _pair_cols(w):
    """[1024, ncols] -> [128, 4, 2, ncols] d-block-pair layout, flattened
    to [128, 4*2*ncols] per-partition-contiguous."""
    n = w.shape[1]
    return np.ascontiguousarray(
        w.reshape(4, 2, 128, n).transpose(2, 0, 1, 3).reshape(128, 8 * n))


def _host_prep(x, Wq, bq, Wk, bk, Wv, bv, Wo, bo, gamma, beta):
    """Build the 8 per-core input dicts."""
    import ml_dtypes
    FP8 = ml_dtypes.float8_e4m3

    x = np.ascontiguousarray(np.asarray(x, np.float32))
    WqR = np.asarray(Wq, np.float32).reshape(D, H * DH)
    WkR = np.asarray(Wk, np.float32).reshape(D, H * DH)
    WvR = np.asarray(Wv, np.float32).reshape(D, H * DH)
    WoR = np.asarray(Wo, np.float32).reshape(H * DH, D)
    bqF = np.asarray(bq, np.float32).reshape(H * DH)
    bkF = np.asarray(bk, np.float32).reshape(H * DH)
    bvF = np.asarray(bv, np.float32).reshape(H * DH)
    boF = np.asarray(bo, np.float32).reshape(D)
    gF = np.asarray(gamma, np.float32).reshape(D)
    btF = np.asarray(beta, np.float32).reshape(D)

    # xT fp8: per window contiguous [128, 4, 2, 512]
    xt8 = []
    for b in range(B):
        xT = x[b].T.astype(FP8)  # [1024, 2048]
        t = (xT.reshape(4, 2, 128, NQB, 512)
             .transpose(3, 2, 0, 1, 4))  # [w, 128, kcp, 2, 512]
        xt8.append(np.ascontiguousarray(t))

    i = np.arange(128)[:, None]
    j = np.arange(128)[None, :]
    maskA = np.where(i >= j, MASKC, 0.0).astype(FP8)  # A[k,p]: k >= p
    eye = np.eye(128, dtype=np.float32)
    cwall = np.concatenate(
        [maskA.astype(np.float32), eye, eye, eye,
         np.ones((128, 32), np.float32),
         np.full((128, 128), MASKC, np.float32),
         np.ones((128, 128), np.float32)], axis=1).astype(FP8)

    biases_zero = not (bqF.any() or bkF.any() or bvF.any())

    in_maps = []
    for c in range(NCORES):
        b, hg = c // 4, c % 4
        cols = slice(256 * hg, 256 * hg + 256)
        wv_c = np.zeros((D, 320), np.float32)
        bv_c = np.zeros((1, 320), np.float32)
        for h2 in range(4):
            wv_c[:, 80 * h2:80 * h2 + 64] = WvR[:, 256 * hg + 64 * h2:
                                                256 * hg + 64 * h2 + 64]
            bv_c[0, 80 * h2:80 * h2 + 64] = bvF[256 * hg + 64 * h2:
                                                256 * hg + 64 * h2 + 64]
        wo_c = np.ascontiguousarray(
            WoR[cols, :].reshape(2, 128, 1024).transpose(1, 0, 2)
            .reshape(128, 2048))
        m = {
            "xt8": xt8[b],
            "wk8": _pair_cols(WkR[:, cols]).astype(FP8),
            "wq8": _pair_cols(WqR[:, cols]).astype(FP8),
            "wv8": _pair_cols(wv_c).astype(FP8),
            "wo8": wo_c.astype(FP8),
            "cwall": cwall,
            "xres": boF[None, :] + np.concatenate(
                [x[b, QB * j2 + 128 * hg:QB * j2 + 128 * hg + 128]
                 for j2 in range(NQB)], axis=0),
        }
        if not biases_zero:
            m["onesr32"] = np.ones((1, 512), np.float32)
            m["bq_r"] = bqF[cols].reshape(1, 256).copy()
            m["bk_r"] = bkF[cols].reshape(1, 256).copy()
            m["bv_r"] = bv_c
        m["gamma_r"] = gF[None, :].copy()
        m["beta_r"] = btF[None, :].copy()
        in_maps.append(m)
    return in_maps


def kernel(**inputs):
    from concourse.bass_utils import run_bass_kernel_spmd

    gamma = np.asarray(inputs["gamma"], np.float32)
    beta = np.asarray(inputs["beta"], np.float32)
    ln_affine = not (np.all(gamma == 1.0) and np.all(beta == 0.0))
    qkv_bias = any(np.asarray(inputs[k], np.float32).any()
                   for k in ("bq", "bk", "bv"))
    nc = _get_program(with_collective=True, ln_affine=ln_affine,
                      qkv_bias=qkv_bias)
    in_maps = _host_prep(**inputs)
    drop = []
    if not ln_affine:
        drop += ["gamma_r", "beta_r"]
    for m in in_maps:
        for k in drop:
            m.pop(k, None)
    res = run_bass_kernel_spmd(nc, in_maps, list(range(NCORES)))
    full = np.empty((B, S, D), np.float32)
    for c in range(NCORES):
        b, r = c // 4, c % 4
        o = res.results[c]["out"]
        for j in range(NQB):
            full[b, QB * j + 128 * r:QB * j + 128 * r + 128, :] = \
                o[128 * j:128 * j + 128]
    return full
